# revision 67
# baseline (speedup 1.0000x reference)
"""Trainium2 Bass kernel for nn_Actor_Critic_GAT_RNN (2-layer GAT + GRU head).

v3 strategy (8 NeuronCores):
  * P2 (GAT layer 1, recomputed per (robot, slot-rank) instance): per-cluster
    PLANAR xe layout (x_j planes contiguous per cluster, even cluster widths)
    so the softmax-numerator tensor_tensor runs in DVE 2x mode; the segmented
    d-reduction is split between the Vector and the otherwise-idle GPSIMD
    engine. e = a_s[src]+a_d[dst] comes from one bf16 matmul per <=31-slot
    chunk (a_d folded via 4 self-x rows in the transposed gather slab).
  * All weight-derived tensors (blkrhs, per-head W1 replication, W2^T|a2
    fusion, GRU weight transposes, bias columns) are precomputed on HOST and
    shipped as two packed blobs -> no on-device weight-prep phase.
  * h1 -> tanh -> W2-row pipeline batched: tile groups of 4 share PSUM via a
    fixed lhsT partition base per slot-rank-mod-4; tanh/G2 copies run on
    [96,384]/[128,408] blocks (rounds of 12 tiles) instead of per-tile.
  * GRU: the sequence (1024 graphs) forgets at ~0.5/step, so each core
    computes ONLY its own 128 steps plus a 64-step warmup. Warmup x moves
    via a pre-masked ReduceScatter (send side multiplied by a per-core
    one-hot so the collective output IS the left neighbor's block; one DMA
    lands it in SX). 6 own-region Picard sweeps overlap the collective,
    then 3 boundary-fix sweeps + 1 full sweep. Outputs are assembled
    host-side from all 8 cores' [128,11] blocks.
"""
import sys

if '/opt/trn_rl_repo' not in sys.path:
    sys.path.insert(0, '/opt/trn_rl_repo')

from contextlib import ExitStack

import numpy as np

import concourse.bass as bass
import concourse.tile as tile
from concourse import bacc, mybir
from concourse.masks import make_identity

F32 = mybir.dt.float32
BF16 = mybir.dt.bfloat16
AL = mybir.AluOpType
AF = mybir.ActivationFunctionType

NCORES = 8
P = 128
H1, HID, F_IN = 6, 16, 4
OUT, FC1, RNN, NACT = 32, 64, 64, 11
T2W = 34
CHUNK_SLOTS = 31              # max slots per e-matmul chunk (31*4+4 = 128 rows)
MAX_NS = 84                   # even; ns*6 <= 504 fp32 (one PSUM bank)
SPLIT = 14                    # reduce planes 0..SPLIT-1 on vector, rest gpsimd
WARM = 64                     # GRU warmup steps
SEQ = WARM + P                # 192 local GRU steps
NITER_PRE = 6                 # own-region sweeps overlapping the AllGather
NITER_FIX = 3                 # post-AG sweeps over the boundary-sensitive part
FIX_END = WARM + 64           # cols covered by the fix sweeps
GTILES = 3                    # tiles per transpose group (lhsT base 0/32/64)
RTILES = 12                   # tiles per tanh/row round (4 groups)


# --------------------------------------------------------------------------
# host-side integer preprocessing
# --------------------------------------------------------------------------

def prepare(edge_index, robot_index, n_nodes):
    src = np.asarray(edge_index[0], dtype=np.int64)
    dst = np.asarray(edge_index[1], dtype=np.int64)
    robots = np.asarray(robot_index, dtype=np.int64)
    N = n_nodes
    B = len(robots)
    assert B == NCORES * P

    indeg = np.bincount(dst, minlength=N)
    deg1 = indeg + 1
    order = np.argsort(dst, kind='stable')
    s_sorted = src[order]
    offs = np.concatenate([[0], np.cumsum(indeg)])

    # L2 slot lists: rank 0 = robot itself, ranks 1+ degree-sorted sources
    rpos = np.full(N, -1, np.int64)
    rpos[robots] = np.arange(B)
    m2 = rpos[dst] >= 0
    s2, d2 = src[m2], dst[m2]
    o2 = np.argsort(rpos[d2], kind='stable')
    s2o = s2[o2]
    cnt2 = np.bincount(rpos[d2], minlength=B)
    off2 = np.concatenate([[0], np.cumsum(cnt2)])
    D2 = int(cnt2.max()) + 1
    slot_node = np.full((B, D2), -1, np.int64)
    slot_node[:, 0] = robots
    for g in range(B):
        srcs = s2o[off2[g]:off2[g + 1]]
        srcs = srcs[np.argsort(-deg1[srcs], kind='stable')]
        slot_node[g, 1:1 + len(srcs)] = srcs
    l2len = cnt2 + 1

    # per-slot-rank max degree (same across cores: one SPMD program)
    validn = slot_node >= 0
    degmat = np.where(validn, deg1[np.maximum(slot_node, 0)], 0)
    Dct = degmat.max(axis=0)                       # [D2]

    # greedy clustering of consecutive ranks with shared Dc (multiple of 4
    # so the pairwise tree-halving reduce stays 4B-aligned)
    ev = lambda v: (v + 3) & ~3
    clusters = []
    t = 0
    while t < D2:
        t0 = t
        mx = ev(int(Dct[t]))
        t += 1
        while t < D2:
            nmx = max(mx, ev(int(Dct[t])))
            if nmx * (t - t0 + 1) > MAX_NS:
                break
            if nmx * (t - t0 + 1) - int(Dct[t0:t + 1].sum()) > 18:
                break
            mx = nmx
            t += 1
        clusters.append(dict(t0=t0, nt=t - t0, Dc=mx))
    # keep rank order: ranks are degree-sorted so big clusters come first
    # anyway, and the h1 tile-groups then complete in emission order
    xe_off = 0
    ch_off = 0
    for cl in clusters:
        cl['ns'] = cl['nt'] * cl['Dc']
        cl['xe_off'] = xe_off
        xe_off += cl['ns']
        chunks = []
        for tl in range(cl['nt']):
            Dc = cl['Dc']
            nch = (Dc + CHUNK_SLOTS - 1) // CHUNK_SLOTS
            base = Dc // nch
            rem = Dc - base * nch
            s0 = 0
            for i in range(nch):
                sc = base + (1 if i < rem else 0)
                chunks.append((tl, s0, sc))
                s0 += sc
        cl['chunks'] = chunks
        cl['ch_off'] = ch_off
        ch_off += len(chunks)
    TOT_SLOTS = xe_off
    TOT_CHUNKS = ch_off

    # host index tables.  PAD row = zeros; QPAD row = the pad-poison vector q
    # (cs.q <= -175 for every head, so exp(leaky(e_pad)) underflows to 0 and
    # den needs no explicit mask).  Invalid slots keep a zero col0 so their
    # den is exp(0)=1 (no inf/NaN from the reciprocal).
    PAD = N
    QPAD = N + 1
    xe_node = np.full((NCORES, P, TOT_SLOTS), PAD, np.int64)
    xeT_fidx = np.full((NCORES, 128, TOT_CHUNKS * 128), PAD * 4, np.int64)
    j4 = np.arange(4)

    for cl in clusters:
        Dc, nt, t0 = cl['Dc'], cl['nt'], cl['t0']
        slots_t = []
        qm_t = []
        for tl in range(nt):
            tt = t0 + tl
            nodes_t = slot_node[:, tt]              # [B]
            vn = nodes_t >= 0
            nn = np.where(vn, nodes_t, 0)
            lens = np.where(vn, indeg[nn], -1)
            starts = offs[nn]
            k = np.arange(Dc - 1)
            gp = starts[:, None] + k[None, :]
            ok = k[None, :] < lens[:, None]
            sl = np.where(ok, s_sorted[np.clip(gp, 0, max(len(s_sorted) - 1, 0))], QPAD)
            col0 = np.where(vn, nodes_t, PAD)
            slots = np.concatenate([col0[:, None], sl], 1)     # [B, Dc]
            slots_t.append(slots)
            c0 = cl['xe_off'] + tl * Dc
            xe_node[:, :, c0:c0 + Dc] = slots.reshape(NCORES, P, Dc)
        slots_all = np.stack(slots_t, 1)            # [B, nt, Dc]
        # transposed chunk layout for the e-matmul
        for ci, (tl, s0, sc) in enumerate(cl['chunks']):
            gch = cl['ch_off'] + ci
            blk = np.full((NCORES, 128, P), PAD * 4, np.int64)
            sA = slots_all[:, tl, s0:s0 + sc]       # [B, sc]
            fT = (sA[:, :, None] * 4 + j4).reshape(B, sc * 4)   # [B, sc*4]
            fT = fT.reshape(NCORES, P, sc * 4).transpose(0, 2, 1)
            blk[:, 0:sc * 4, :] = fT
            selfn = slots_all[:, tl, 0]             # instance node (self slot)
            selfF = (np.minimum(selfn, N)[:, None] * 4 + j4) \
                .reshape(NCORES, P, 4).transpose(0, 2, 1)
            blk[:, 124:128, :] = selfF
            xeT_fidx[:, :, gch * 128:(gch + 1) * 128] = blk

    mask_l2 = np.where(np.arange(D2)[None, :] < l2len[:, None], 0.0, -100.0)
    mask_l2 = mask_l2.reshape(NCORES, P, D2).astype(np.float32)

    # tile -> processing position of the cluster that produces it
    tile_cluster = np.zeros(D2, np.int64)
    for ci, cl in enumerate(clusters):
        tile_cluster[cl['t0']:cl['t0'] + cl['nt']] = ci

    cfg = dict(N=N, B=B, D2=D2, TOT_SLOTS=TOT_SLOTS, TOT_CHUNKS=TOT_CHUNKS,
               clusters=clusters, tile_cluster=tile_cluster.tolist())
    tables = dict(xe_node=xe_node, xeT_fidx=xeT_fidx, mask_l2=mask_l2)
    return cfg, tables


# --------------------------------------------------------------------------
# AP helpers
# --------------------------------------------------------------------------

def vap(view, *dims):
    return bass.AP(view.tensor, view.offset,
                   [list(view.ap[0])] + [list(d) for d in dims])


def offap(view, extra_offset, *dims):
    return bass.AP(view.tensor, view.offset + extra_offset,
                   [list(view.ap[0])] + [list(d) for d in dims])


# --------------------------------------------------------------------------
# device program
# --------------------------------------------------------------------------

def build_program(cfg, debug=False, taps=False):
    D2 = cfg['D2']
    TOT_SLOTS = cfg['TOT_SLOTS']
    TOT_CHUNKS = cfg['TOT_CHUNKS']
    clusters = cfg['clusters']
    tile_cluster = cfg['tile_cluster']

    NWB = 647                 # bf16 weight blob cols
    NWF = 52 + D2             # f32 weight blob cols

    nc = bacc.Bacc("TRN2", target_bir_lowering=False, debug=debug,
                   num_devices=NCORES)

    def inp(name, shape, dtype=F32):
        return nc.dram_tensor(name, list(shape), dtype, kind="ExternalInput").ap()

    xe_d = inp("x_edge", (P, TOT_SLOTS * 4), BF16)
    xeT_d = inp("x_edge_T", (128, TOT_CHUNKS * 128), BF16)
    wb_d = inp("wblob_bf", (128, NWB), BF16)
    wf_d = inp("wblob_f32", (128, NWF))
    out_d = nc.dram_tensor("out", [P, NACT], F32, kind="ExternalOutput").ap()

    rs_in = nc.dram_tensor("rs_in", [NCORES * RNN, WARM], BF16).ap()
    rs_out = nc.dram_tensor("rs_out", [RNN, WARM], BF16).ap()
    warm_in = nc.dram_tensor("warm_in", [1, 64], BF16).ap()
    warm_out = nc.dram_tensor("warm_out", [NCORES, 64], BF16,
                              addr_space="Shared").ap()
    rg = [list(range(NCORES))]

    NG = (D2 + GTILES - 1) // GTILES           # transpose groups
    # tanh/row round boundaries (multiples of GTILES); the final round is
    # split so the post-last-cluster serial tail is short
    RB = list(range(0, D2, RTILES)) + [D2]
    if RB[-1] - RB[-2] > 6:
        mid = RB[-2] + ((RB[-1] - RB[-2]) // 2 + GTILES - 1) // GTILES * GTILES
        RB.insert(-1, mid)
    NR = len(RB) - 1

    with tile.TileContext(nc) as tc, ExitStack() as ctx:
        const = ctx.enter_context(tc.tile_pool(name="const", bufs=1))

        # warm up the CC stream so the real AllGather doesn't pay the
        # first-collective initialization cost
        warmt = const.tile([1, 64], BF16)
        nc.vector.memset(warmt[:], 0.0)
        nc.sync.dma_start(out=warm_in[:, :], in_=warmt[:])
        nc.gpsimd.collective_compute(
            "AllGather", AL.bypass, replica_groups=rg,
            ins=[warm_in[:, :]], outs=[warm_out[:, :]])

        # weight blobs FIRST: the first e-matmul needs blkrhs, so these must
        # not queue behind the big slab DMAs
        wb = const.tile([128, NWB], BF16)
        nc.sync.dma_start(out=wb[:], in_=wb_d[:])
        wf = const.tile([128, NWF], F32)
        nc.sync.dma_start(out=wf[:], in_=wf_d[:])

        # big input slabs, DMA'd per cluster so the first e-matmul only
        # waits for its own slice; xeT on the scalar HWDGE queue so xe and
        # xeT stream in parallel
        xe_all = const.tile([P, TOT_SLOTS * 4], BF16)
        xeT_all = const.tile([128, TOT_CHUNKS * 128], BF16)
        for cl in clusters:
            c0 = cl['ch_off'] * 128
            c1 = (cl['ch_off'] + len(cl['chunks'])) * 128
            nc.sync.dma_start(out=xeT_all[:, c0:c1], in_=xeT_d[:, c0:c1])
            s0 = cl['xe_off'] * 4
            s1 = (cl['xe_off'] + cl['ns']) * 4
            nc.sync.dma_start(out=xe_all[:, s0:s1], in_=xe_d[:, s0:s1])

        # blob views
        w1rep = wb[:, 0:96]
        blkrhs = wb[:, 96:282]
        Wr = wb[:, 282:346]
        Wz = wb[:, 346:410]
        fc1T = wb[0:36, 410:474]
        rhs2 = wb[0:96, 474:508]
        WhnT = wb[0:RNN, 508:572]
        WinT64 = wb[RNN:128, 572:636]     # WinT replicated at partitions 64:128
        fc2T = wb[0:RNN, 636:647]
        c1b = wf[0:96, 0:1]
        fc1b = wf[0:FC1, 1:2]
        brz = wf[:, 2:3]
        brzneg = wf[:, 3:4]
        bnih = wf[0:RNN, 4:5]
        bnhh = wf[0:RNN, 5:6]
        fc2b = wf[0:NACT, 6:7]
        selv = wf[0:RNN, 7:15]
        m0 = wf[0:RNN, 15:16]
        c2b_rep = wf[:, 16:48]
        maskl2 = wf[:, 48:48 + D2]
        rf = wf[:, 48 + D2:52 + D2]

        ident = const.tile([P, P], F32)
        make_identity(nc, ident[:])
        identb = const.tile([P, P], BF16)
        nc.vector.tensor_copy(out=identb[:], in_=ident[:])

        G2 = const.tile([P, D2 * T2W], BF16)
        aggball = const.tile([P, D2 * 32], BF16)
        aggf = const.tile([P, 30 * D2], BF16)   # (j,h)-plane major, t minor
        p2t_all = const.tile([P, D2], BF16)
        hsum_acc = const.tile([P, OUT], F32)
        nc.vector.memset(hsum_acc[:], 0.0)

        # ---- P2: layer-1 instances -> G2 (SBUF resident) ----
        p2ctx = ExitStack()
        p2 = p2ctx.enter_context(tc.tile_pool(name="p2", bufs=2))
        epsp = p2ctx.enter_context(tc.tile_pool(name="epsp", bufs=2, space="PSUM"))
        t4p = p2ctx.enter_context(tc.tile_pool(name="t4p", bufs=1, space="PSUM"))
        h1p = p2ctx.enter_context(tc.tile_pool(name="h1p", bufs=1, space="PSUM"))
        rowp = p2ctx.enter_context(tc.tile_pool(name="rowp", bufs=2, space="PSUM"))

        HCOLS = (RTILES // GTILES) * 128
        h1ps = [h1p.tile([96, HCOLS], F32, space="PSUM", tag=f"h1_{b}",
                         name=f"h1ps{b}")
                for b in range(GTILES)]
        h1T = [const.tile([96, HCOLS], BF16, name=f"h1T{b}")
               for b in range(GTILES)]

        rounds_emitted = [0]

        def tile_ready(tg, ncl):
            return tile_cluster[tg] < ncl

        def emit_round(r):
            tlo = RB[r]
            thi = RB[r + 1]
            ntr = thi - tlo
            ngr = (ntr + GTILES - 1) // GTILES
            # alpha normalization for this round's tiles (batched)
            recd = p2.tile([P, 6 * RTILES], BF16, tag="rec")
            with nc.allow_low_precision(reason="bf16 alpha normalization"):
                nc.vector.reciprocal(
                    out=vap(recd[:], [ntr, 6], [1, ntr]),
                    in_=offap(aggf[:], 24 * D2 + tlo, [D2, 6], [1, ntr]))
            nc.vector.tensor_tensor(
                out=offap(aggball[:], tlo * 32, [32, ntr], [4, 6], [1, 4]),
                in0=offap(aggf[:], tlo, [1, ntr], [D2, 6], [6 * D2, 4]),
                in1=vap(recd[:], [1, ntr], [ntr, 6], [0, 4]),
                op=AL.mult)
            for gl in range(ngr):
                glo = tlo + gl * GTILES
                ghi = min(glo + GTILES, thi)
                ncols = (ghi - glo) * 32
                T4ps = t4p.tile([128, 128], BF16, space="PSUM", tag="t4")
                nc.tensor.transpose(out=T4ps[0:ncols, :],
                                    in_=aggball[:, glo * 32:ghi * 32],
                                    identity=identb[:])
                aggT4 = p2.tile([128, 128], BF16, tag="aggT")
                nc.scalar.activation(out=aggT4[0:ncols, :], in_=T4ps[0:ncols, :],
                                     func=AF.Copy)
                for b in range(ghi - glo):
                    nc.tensor.matmul(
                        out=h1ps[b][:, gl * 128:gl * 128 + 128],
                        lhsT=w1rep[32 * b:32 * b + 24, :],
                        rhs=aggT4[32 * b:32 * b + 24, :], start=True, stop=True)
            for b in range(GTILES):
                cols = ngr * 128
                nc.scalar.activation(out=h1T[b][:, 0:cols],
                                     in_=h1ps[b][:, 0:cols],
                                     func=AF.Tanh, bias=c1b[:, 0:1])
            row_ps = rowp.tile([P, RTILES * T2W], F32, space="PSUM", tag="row")
            for tg in range(tlo, thi):
                rc = tg - tlo
                b = tg % GTILES
                gc = rc // GTILES
                nc.tensor.matmul(
                    out=row_ps[:, rc * T2W:(rc + 1) * T2W],
                    lhsT=h1T[b][:, gc * 128:gc * 128 + 128],
                    rhs=rhs2[:], start=True, stop=True)
            nc.scalar.activation(
                out=G2[:, tlo * T2W:thi * T2W],
                in_=row_ps[:, 0:(thi - tlo) * T2W], func=AF.Copy)
            # incremental layer-2 softmax pieces (hidden under later clusters)
            ntr = thi - tlo
            e2r = p2.tile([P, RTILES], F32, tag="e2r")
            nc.vector.tensor_add(
                out=e2r[:, 0:ntr],
                in0=offap(G2[:], tlo * T2W + 32, [T2W, ntr]),
                in1=offap(G2[:], 33, [0, ntr]))
            nc.vector.tensor_add(out=e2r[:, 0:ntr], in0=e2r[:, 0:ntr],
                                 in1=maskl2[:, tlo:thi])
            nc.vector.scalar_tensor_tensor(out=e2r[:, 0:ntr], in0=e2r[:, 0:ntr],
                                           scalar=0.2, in1=e2r[:, 0:ntr],
                                           op0=AL.mult, op1=AL.max)
            nc.scalar.activation(out=p2t_all[:, tlo:thi], in_=e2r[:, 0:ntr],
                                 func=AF.Exp)
            num2r = p2.tile([P, RTILES * OUT], BF16, tag="num2r")
            nc.vector.tensor_tensor(
                out=vap(num2r[:], [ntr, OUT], [1, ntr]),
                in0=vap(p2t_all[:, tlo:thi], [0, OUT], [1, ntr]),
                in1=offap(G2[:], tlo * T2W, [1, OUT], [T2W, ntr]),
                op=AL.mult)
            hsr = p2.tile([P, OUT], F32, tag="hsr")
            nc.vector.tensor_reduce(
                out=hsr[:], in_=vap(num2r[:], [ntr, OUT], [1, ntr]),
                axis=mybir.AxisListType.X, op=AL.add)
            nc.vector.tensor_add(out=hsum_acc[:], in0=hsum_acc[:], in1=hsr[:])

        for ncl, cl in enumerate(clusters):
            nt, Dc, t0, ns = cl['nt'], cl['Dc'], cl['t0'], cl['ns']
            xe_v = xe_all[:, cl['xe_off'] * 4:(cl['xe_off'] + ns) * 4]
            eps = epsp.tile([P, ns * 6], F32, space="PSUM", tag="eps")
            for ci, (tl, s0, sc) in enumerate(cl['chunks']):
                gch = cl['ch_off'] + ci
                col0 = (tl * Dc + s0) * 6
                nc.tensor.matmul(
                    out=eps[:, col0:col0 + sc * 6],
                    lhsT=xeT_all[:, gch * 128:(gch + 1) * 128],
                    rhs=blkrhs[:, 0:sc * 6], start=True, stop=True)
            # leaky relu (PSUM -> SBUF bf16), layout (t, d, h)
            e_t = p2.tile([P, ns * 6], BF16, tag="e")
            nc.scalar.activation(out=e_t[:], in_=eps[:], func=AF.Prelu,
                                 alpha=0.2)
            # exp with (t,d,h) -> (h, t*d) layout change
            pt = p2.tile([P, ns * 6], BF16, tag="pt")
            nc.scalar.activation(
                out=vap(pt[:], [ns, 6], [1, ns]),
                in_=vap(e_t[:], [1, 6], [6, ns]),
                func=AF.Exp)
            # num24 (j, h, t*d) = xe planes (j, t*d) * pt (h, t*d); all
            # inner strides 1 and bf16 -> DVE 2x mode
            num24 = p2.tile([P, ns * 24], BF16, tag="num")
            nc.vector.tensor_tensor(
                out=vap(num24[:], [6 * ns, 4], [ns, 6], [1, ns]),
                in0=vap(pt[:], [0, 4], [ns, 6], [1, ns]),
                in1=vap(xe_v, [ns, 4], [0, 6], [1, ns]),
                op=AL.mult)
            # two bf16 pairwise-halving levels (2x mode) + fp32 tail reduce
            d2, d4 = Dc // 2, Dc // 4
            numh = p2.tile([P, ns * 12], BF16, tag="numh")
            nc.vector.tensor_tensor(
                out=vap(numh[:], [ns // 2, 24], [d2, nt], [1, d2]),
                in0=vap(num24[:], [ns, 24], [Dc, nt], [1, d2]),
                in1=offap(num24[:], d2, [ns, 24], [Dc, nt], [1, d2]),
                op=AL.add)
            numq = p2.tile([P, ns * 6], BF16, tag="numq")
            nc.vector.tensor_tensor(
                out=vap(numq[:], [ns // 4, 24], [d4, nt], [1, d4]),
                in0=vap(numh[:], [ns // 2, 24], [d2, nt], [1, d4]),
                in1=offap(numh[:], d4, [ns // 2, 24], [d2, nt], [1, d4]),
                op=AL.add)
            with nc.allow_low_precision(reason="bf16 agg after fp32-internal reduce"):
                nc.vector.tensor_reduce(
                    out=offap(aggf[:], t0, [D2, 24], [1, nt]),
                    in_=vap(numq[:], [ns // 4, 24], [d4, nt], [1, d4]),
                    axis=mybir.AxisListType.X, op=AL.add)
            # den = sum over d of pt (pads underflow to 0 via the q-poison);
            # same halving tree, reading pt in place
            dnh = p2.tile([P, ns * 3], BF16, tag="dnh")
            nc.vector.tensor_tensor(
                out=vap(dnh[:], [ns // 2, 6], [d2, nt], [1, d2]),
                in0=vap(pt[:], [ns, 6], [Dc, nt], [1, d2]),
                in1=offap(pt[:], d2, [ns, 6], [Dc, nt], [1, d2]),
                op=AL.add)
            dnq = p2.tile([P, ns * 3 // 2], BF16, tag="dnq")
            nc.vector.tensor_tensor(
                out=vap(dnq[:], [ns // 4, 6], [d4, nt], [1, d4]),
                in0=vap(dnh[:], [ns // 2, 6], [d2, nt], [1, d4]),
                in1=offap(dnh[:], d4, [ns // 2, 6], [d2, nt], [1, d4]),
                op=AL.add)
            with nc.allow_low_precision(reason="bf16 den after fp32-internal reduce"):
                nc.vector.tensor_reduce(
                    out=offap(aggf[:], 24 * D2 + t0, [D2, 6], [1, nt]),
                    in_=vap(dnq[:], [ns // 4, 6], [d4, nt], [1, d4]),
                    axis=mybir.AxisListType.X, op=AL.add)
            # fire any rounds whose tiles are all written
            while rounds_emitted[0] < NR:
                r = rounds_emitted[0]
                tlo = RB[r]
                thi = RB[r + 1]
                if not all(tile_ready(t, ncl + 1) for t in range(tlo, thi)):
                    break
                emit_round(r)
                rounds_emitted[0] += 1

        p2ctx.close()

        # ---- P3: finish layer-2 softmax + fc1 ----
        p3ctx = ExitStack()
        p3 = p3ctx.enter_context(tc.tile_pool(name="p3", bufs=1))
        p3ps = p3ctx.enter_context(tc.tile_pool(name="p3ps", bufs=2, space="PSUM"))
        den2 = p3.tile([P, 1], F32)
        nc.vector.tensor_reduce(out=den2[:], in_=p2t_all[:],
                                axis=mybir.AxisListType.X, op=AL.add)
        rec2 = p3.tile([P, 1], F32)
        nc.vector.reciprocal(out=rec2[:], in_=den2[:])
        Z = p3.tile([P, 36], F32)
        nc.vector.scalar_tensor_tensor(out=Z[:, 0:32], in0=hsum_acc[:],
                                       scalar=rec2[:, 0:1], in1=c2b_rep,
                                       op0=AL.mult, op1=AL.add)
        nc.vector.tensor_copy(out=Z[:, 32:36], in_=rf)
        ZT_ps = p3ps.tile([36, P], F32, space="PSUM")
        nc.tensor.transpose(out=ZT_ps[:], in_=Z[:], identity=ident[:])
        ZT = p3.tile([36, P], BF16)
        nc.vector.tensor_copy(out=ZT[:], in_=ZT_ps[:])
        XT_ps = p3ps.tile([RNN, P], F32, space="PSUM")
        nc.tensor.matmul(out=XT_ps[:], lhsT=fc1T[:], rhs=ZT[:],
                         start=True, stop=True)
        xt_sb = const.tile([RNN, P], BF16)
        nc.scalar.activation(out=xt_sb[:], in_=XT_ps[:], func=AF.Tanh,
                             bias=fc1b[:, 0:1])
        p3ctx.close()

        if taps:
            dbg_g2 = nc.dram_tensor("dbg_g2", [P, D2 * T2W], BF16,
                                    kind="ExternalOutput").ap()
            nc.sync.dma_start(out=dbg_g2[:, :], in_=G2[:])
            dbg_x = nc.dram_tensor("dbg_x", [RNN, P], BF16,
                                   kind="ExternalOutput").ap()
            nc.sync.dma_start(out=dbg_x[:, :], in_=xt_sb[:])

        # ship the second-half x columns (warmup for the next core)
        # pre-masked ReduceScatter: block r of the send buffer is our
        # second-half x iff r == (core+1) % 8, zero otherwise -> each core
        # RECEIVES exactly its left neighbor's warmup block, no combine
        rsb = const.tile([RNN, NCORES * WARM], BF16)
        nc.vector.tensor_tensor(
            out=vap(rsb[:], [WARM, NCORES], [1, WARM]),
            in0=offap(xt_sb[:], WARM, [0, NCORES], [1, WARM]),
            in1=vap(selv, [1, NCORES], [0, WARM]), op=AL.mult)
        nc.sync.dma_start(
            out=bass.AP(rs_in.tensor, 0,
                        [[WARM, RNN], [RNN * WARM, NCORES], [1, WARM]]),
            in_=vap(rsb[:], [WARM, NCORES], [1, WARM]))
        nc.gpsimd.collective_compute(
            "ReduceScatter", AL.add, replica_groups=rg,
            ins=[rs_in[:, :]], outs=[rs_out[:, :]])

        # ---- P4: local GRU (WARM warmup + P own steps) ----
        gru = ctx.enter_context(tc.tile_pool(name="gru", bufs=2))
        grup = ctx.enter_context(tc.tile_pool(name="grup", bufs=2, space="PSUM"))
        # SX [128, SEQ+1]: rows 0:64 h (col j = h_{j-1}; col 0 = 0),
        # rows 64:128 x (col j = x_j; col SEQ unused)
        SX = gru.tile([128, SEQ + 1], BF16, tag="SX")
        nc.vector.memset(SX[:], 0.0)
        nc.scalar.activation(out=SX[RNN:128, WARM:SEQ], in_=xt_sb[:, 0:P],
                             func=AF.Copy)
        GIn = gru.tile([RNN, SEQ], BF16, tag="GIn")
        nc.vector.memset(GIn[:, 0:WARM], 0.0)
        gi_ps = grup.tile([RNN, P], F32, space="PSUM", tag="psB")
        nc.tensor.matmul(out=gi_ps[:], lhsT=WinT64[:],
                         rhs=SX[RNN:128, WARM:SEQ], start=True, stop=True)
        nc.scalar.activation(out=GIn[:, WARM:SEQ], in_=gi_ps[:], func=AF.Copy)

        def sweep(c0, c1):
            L = c1 - c0
            # r and z as column blocks of one [64, 2*L] PSUM tile so the
            # whole gate path stays on partitions 0:64 (no partition rebase)
            rz_ps = grup.tile([RNN, 2 * SEQ], F32, space="PSUM", tag="psA")
            n_ps = grup.tile([RNN, SEQ], F32, space="PSUM", tag="psB")
            nc.tensor.matmul(out=rz_ps[:, 0:L], lhsT=Wr[:],
                             rhs=SX[:, c0:c1], start=True, stop=True)
            nc.tensor.matmul(out=rz_ps[:, L:2 * L], lhsT=Wz[:],
                             rhs=SX[:, c0:c1], start=True, stop=True)
            nc.tensor.matmul(out=n_ps[:, 0:L], lhsT=WhnT[:],
                             rhs=SX[0:RNN, c0:c1], start=True, stop=True)
            RZ = gru.tile([RNN, 2 * SEQ], BF16, tag="RZ")
            zc = gru.tile([RNN, SEQ], BF16, tag="zc")
            # R first so the vector u-chain starts as early as possible
            nc.scalar.activation(out=RZ[:, 0:L], in_=rz_ps[:, 0:L],
                                 func=AF.Sigmoid, bias=brz[0:RNN, 0:1])
            nc.scalar.activation(out=RZ[:, L:2 * L], in_=rz_ps[:, L:2 * L],
                                 func=AF.Sigmoid, bias=brz[0:RNN, 0:1])
            nc.scalar.activation(out=zc[:, 0:L], in_=rz_ps[:, L:2 * L],
                                 func=AF.Sigmoid, scale=-1.0,
                                 bias=brzneg[0:RNN, 0:1])
            u = gru.tile([RNN, SEQ], BF16, tag="u")
            nc.vector.scalar_tensor_tensor(out=u[:, 0:L], in0=n_ps[:, 0:L],
                                           scalar=bnhh[:, 0:1],
                                           in1=RZ[:, 0:L],
                                           op0=AL.add, op1=AL.mult)
            nc.vector.tensor_add(out=u[:, 0:L], in0=u[:, 0:L],
                                 in1=GIn[:, c0:c1])
            Nt = gru.tile([RNN, SEQ], BF16, tag="Nt")
            nc.scalar.activation(out=Nt[:, 0:L], in_=u[:, 0:L], func=AF.Tanh,
                                 bias=bnih[:, 0:1])
            Mp = gru.tile([RNN, SEQ], BF16, tag="Mp")
            nc.vector.tensor_mul(out=Mp[:, 0:L], in0=zc[:, 0:L],
                                 in1=Nt[:, 0:L])
            if c0 == 0:
                # core 0: zero m over warmup cols so h stays 0 until t=0
                nc.vector.tensor_scalar(out=Mp[:, 0:WARM], in0=Mp[:, 0:WARM],
                                        scalar1=m0[:, 0:1], scalar2=None,
                                        op0=AL.mult)
            nc.vector.tensor_tensor_scan(
                out=SX[0:RNN, c0 + 1:c1 + 1], data0=RZ[:, L:2 * L],
                data1=Mp[:, 0:L], initial=0.0, op0=AL.mult, op1=AL.add)

        for _ in range(NITER_PRE):
            sweep(WARM, SEQ)

        # the ReduceScatter output IS the left neighbor's warmup block:
        # one DMA straight into SX (sync queue; no compute-engine FIFO wait)
        nc.sync.dma_start(out=SX[RNN:128, 0:WARM], in_=rs_out[:, :])
        gi2_ps = grup.tile([RNN, WARM], F32, space="PSUM", tag="psB")
        nc.tensor.matmul(out=gi2_ps[:], lhsT=WinT64[:],
                         rhs=SX[RNN:128, 0:WARM], start=True, stop=True)
        nc.scalar.activation(out=GIn[:, 0:WARM], in_=gi2_ps[:], func=AF.Copy)

        for _ in range(NITER_FIX):
            sweep(0, FIX_END)
        sweep(0, SEQ)

        if taps:
            dbg_sx = nc.dram_tensor("dbg_sx", [128, SEQ + 1], BF16,
                                    kind="ExternalOutput").ap()
            nc.sync.dma_start(out=dbg_sx[:, :], in_=SX[:])

        # ---- P5: fc2 + transpose out (own 128 steps = cols WARM+1..SEQ) ----
        l_ps = grup.tile([NACT, P], F32, space="PSUM", tag="psB")
        nc.tensor.matmul(out=l_ps[:], lhsT=fc2T[:],
                         rhs=SX[0:RNN, WARM + 1:SEQ + 1], start=True, stop=True)
        lt = gru.tile([NACT, P], F32, tag="lt")
        nc.scalar.activation(out=lt[:], in_=l_ps[:], func=AF.Identity,
                             bias=fc2b[:, 0:1])
        o_ps = grup.tile([P, NACT], F32, space="PSUM", tag="psA")
        nc.tensor.transpose(out=o_ps[:], in_=lt[:], identity=ident[:NACT, :NACT])
        osb = gru.tile([P, NACT], F32, tag="osb")
        nc.vector.tensor_copy(out=osb[:], in_=o_ps[:])
        nc.sync.dma_start(out=out_d[:, :], in_=osb[:])

    nc.compile()
    return nc


# --------------------------------------------------------------------------
# host-side input packing
# --------------------------------------------------------------------------

def make_in_maps(cfg, tables, inputs):
    N = cfg['N']
    D2 = cfg['D2']
    TOT = cfg['TOT_SLOTS']
    f32 = lambda a: np.ascontiguousarray(a, dtype=np.float32)
    bfnp = mybir.dt.np(BF16)
    x = f32(inputs['x'])

    # ---- weight-derived tensors (host) ----
    c1W = f32(inputs['c1_W'])                    # [96, 4]
    a_s = f32(inputs['c1_as'])                   # [6, 16]
    a_d = f32(inputs['c1_ad'])
    W1h = c1W.reshape(H1, HID, F_IN)
    cs = np.einsum('hf,hfj->hj', a_s, W1h)       # [6, 4]
    cd = np.einsum('hf,hfj->hj', a_d, W1h)

    # pad-poison vector: cs.q <= -200 for every head so exp(leaky(e_pad))
    # underflows to 0 and den needs no mask plane
    q = np.linalg.lstsq(cs, -np.ones(H1, np.float32), rcond=None)[0]
    vq = cs @ q
    q = q * (200.0 / max(-vq.max(), 1e-6))
    assert (cs @ q).max() <= -150.0, "pad-poison vector infeasible"

    # x rows: [x, zeros (PAD), q (QPAD)]
    xpad_bf = np.concatenate(
        [x, np.zeros((1, 4), np.float32), q[None, :]]).astype(bfnp)
    xqflat_bf = xpad_bf.reshape(-1)

    blkrhs = np.zeros((128, 6 * CHUNK_SLOTS), np.float32)
    for s in range(CHUNK_SLOTS):
        for j in range(4):
            blkrhs[4 * s + j, s * 6:(s + 1) * 6] = cs[:, j]
    for j in range(4):
        for s in range(CHUNK_SLOTS):
            blkrhs[124 + j, s * 6:(s + 1) * 6] = cd[:, j]

    w1rep = np.zeros((128, 96), np.float32)
    for b0 in range(0, 128, 32):
        for h in range(H1):
            for j in range(4):
                w1rep[b0 + 4 * h + j, 16 * h:16 * (h + 1)] = W1h[h, :, j]

    c2W = f32(inputs['c2_W'])                    # [32, 96]
    rhs2 = np.zeros((96, T2W), np.float32)
    rhs2[:, 0:32] = c2W.T
    rhs2[:, 32] = c2W.T @ f32(inputs['c2_as'])[0]
    rhs2[:, 33] = c2W.T @ f32(inputs['c2_ad'])[0]

    fc1T = f32(inputs['fc1_W']).T                # [36, 64]
    wih = f32(inputs['gru_wih'])                 # [192, 64]
    whh = f32(inputs['gru_whh'])                 # [192, 64]
    Wr = np.concatenate([whh[0:64].T, wih[0:64].T], axis=0)      # [128, 64]
    Wz = np.concatenate([whh[64:128].T, wih[64:128].T], axis=0)  # [128, 64]
    WhnT = whh[128:192].T                        # [64, 64]
    WinT = wih[128:192].T
    bih = f32(inputs['gru_bih'])
    bhh = f32(inputs['gru_bhh'])
    fc2T = f32(inputs['fc2_W']).T                # [64, 11]

    NWB = 647
    wblob = np.zeros((128, NWB), np.float32)
    wblob[:, 0:96] = w1rep
    wblob[:, 96:282] = blkrhs
    wblob[:, 282:346] = Wr
    wblob[:, 346:410] = Wz
    wblob[0:36, 410:474] = fc1T
    wblob[0:96, 474:508] = rhs2
    wblob[0:RNN, 508:572] = WhnT
    wblob[0:RNN, 572:636] = WinT
    wblob[RNN:128, 572:636] = WinT          # replica at partitions 64:128
    wblob[0:RNN, 636:647] = fc2T
    wblob_bf = wblob.astype(bfnp)

    NWF = 52 + D2
    wf_base = np.zeros((128, NWF), np.float32)
    wf_base[0:96, 0] = f32(inputs['c1_b'])
    wf_base[0:FC1, 1] = f32(inputs['fc1_b'])
    wf_base[:, 2] = (bih + bhh)[0:128]
    wf_base[:, 3] = -(bih + bhh)[0:128]
    wf_base[0:RNN, 4] = bih[128:192]
    wf_base[0:RNN, 5] = bhh[128:192]
    wf_base[0:NACT, 6] = f32(inputs['fc2_b'])
    wf_base[:, 16:48] = f32(inputs['c2_b'])[None, :]

    rfs = f32(inputs['robot_features'])
    clusters = cfg['clusters']

    in_maps = []
    for c in range(NCORES):
        m = dict(wblob_bf=wblob_bf)
        # planar xe: per cluster [4 planes (x0..x3), ns] local layout
        xn = tables['xe_node'][c]                # [P, TOT]
        xg = xpad_bf[np.minimum(xn, N)]          # [P, TOT, 4]
        xe4 = np.empty((P, TOT * 4), bfnp)
        for cl in clusters:
            o, ns = cl['xe_off'], cl['ns']
            blkx = xg[:, o:o + ns, :].transpose(0, 2, 1)     # [P, 4, ns]
            xe4[:, 4 * o:4 * (o + ns)] = blkx.reshape(P, 4 * ns)
        m['x_edge'] = np.ascontiguousarray(xe4)
        m['x_edge_T'] = np.ascontiguousarray(xqflat_bf[tables['xeT_fidx'][c]])
        wfc = wf_base.copy()
        wfc[0:RNN, 7 + (c + 1) % NCORES] = 1.0   # send one-hot (block c+1)
        if c > 0:
            wfc[0:RNN, 15] = 1.0                 # m0 (core 0 zeroes warmup m)
        wfc[:, 48:48 + D2] = tables['mask_l2'][c]
        wfc[:, 48 + D2:52 + D2] = rfs[c * P:(c + 1) * P]
        m['wblob_f32'] = np.ascontiguousarray(wfc)
        in_maps.append(m)
    return in_maps


_CACHE = {}


def kernel(**inputs):
    from concourse import bass_utils
    N = inputs['x'].shape[0]
    cfg, tables = prepare(inputs['edge_index'], inputs['robot_index'], N)
    key = (N, cfg['B'], cfg['D2'], cfg['TOT_SLOTS'],
           tuple((cl['t0'], cl['nt'], cl['Dc']) for cl in cfg['clusters']))
    if key not in _CACHE:
        _CACHE[key] = build_program(cfg, debug=False)
    nc = _CACHE[key]
    in_maps = make_in_maps(cfg, tables, inputs)
    res = bass_utils.run_bass_kernel_spmd(nc, in_maps, core_ids=list(range(NCORES)))
    return np.concatenate(
        [np.asarray(res.results[c]['out'], dtype=np.float32)
         for c in range(NCORES)], axis=0)
